# revision 32
# baseline (speedup 1.0000x reference)
"""AttentionPairBias kernel for 8 Trainium2 NeuronCores.

Sharding: rows of the query sequence (S=1024) are split across the 8 cores
(128 rows each). The pair tensor z's bias contribution, the softmax and the
output rows are all embarrassingly parallel in the query dimension, so no
collectives are needed; each core reads its own 128x1024x128 slice of z.

Per-core pipeline (v2 — fp8 DoubleRow pair stream):
  1. z arrives host-transposed as [c=128, row, t] fp8(e4m3); plain DMA loads
     (no XBAR transpose).  Squares z^2 are computed on-device (DVE/ACT/Pool
     rotation) into the second DoubleRow slice.
  2. One fp8 DoubleRow matmul per (row, t-chunk) contracts c over both
     slices at once: slice0 weights = 64*(ln_w*Wz - c1/DZ | 1/DZ | 0),
     slice1 weights = 64*(0 | .. | 1/DZ) -> y'[h], 64*mu, 64*E[z^2] in one
     PSUM pass (the c1 mean-fold is baked into the weights; ln_b dropped:
     softmax-invariant).  4 rows pack one 2-bank PSUM tile via col-tiling.
  3. PSUM -> bf16 y4 copy (DVE/ACT alternate), one DMA per 4-row group to a
     flat DRAM scratch y_flat[32*row + m, t]; per-head tiles read back with
     a uniform-stride gather y_flat[h::32].
  4. var*4096 = 64*ez2_s - mu_s^2; r/64 = rsqrt-via-Ln/Exp(var4096+4096eps);
     bias_h = (r/64)*y'_s.
  5. Per head: scores = qk/sqrt(hd) + bias -> PE transpose -> exp on ACT
     (max-subtraction-free: |scores| < 4) -> A@[V|1] gives o and the softmax
     denominator in one accumulation chain.
  6. sigmoid gate, output projection.
"""

import os
import sys
import types
import numpy as np

for _p in ("/opt/trn_rl_repo", "/root/.axon_site/_ro/trn_rl_repo"):
    if os.path.isdir(_p) and _p not in sys.path:
        sys.path.append(_p)

import ml_dtypes
from contextlib import ExitStack

import concourse.bass as bass
import concourse.mybir as mybir
import concourse.tile as tile
from concourse import bacc
from concourse.bass import ds, ts
from concourse.masks import make_identity

F8 = mybir.dt.float8e4
BF16 = mybir.dt.bfloat16
FP32 = mybir.dt.float32
AF = mybir.ActivationFunctionType
ALU = mybir.AluOpType
DR = mybir.MatmulPerfMode.DoubleRow

S = 1024
D = 768
H = 16
HD = 48
HDP = 64            # padded head dim (2 heads per 128-partition block)
DP = H * HDP        # 1024
DZ = 128
EPS = 1e-5
N_CORES = 8
RPC = S // N_CORES  # 128 rows per core
ISQ = float(HD) ** -0.5
WS = 64.0           # fp8 weight scale
EPS_S = EPS * WS * WS

_CACHE = {}


def _build():
    nc = bacc.Bacc("TRN2", target_bir_lowering=False, debug=False,
                   num_devices=N_CORES)

    zb = nc.dram_tensor("zb", [DZ, RPC, S], F8, kind="ExternalInput").ap()
    w01 = nc.dram_tensor("w01", [DZ, 2, 32], F8, kind="ExternalInput").ap()
    sTc = nc.dram_tensor("sTc", [D, RPC], BF16, kind="ExternalInput").ap()
    WqT = nc.dram_tensor("WqT", [D, DP], BF16, kind="ExternalInput").ap()
    WkT = nc.dram_tensor("WkT", [D, DP], BF16, kind="ExternalInput").ap()
    WvT = nc.dram_tensor("WvT", [D, DP], BF16, kind="ExternalInput").ap()
    WgT = nc.dram_tensor("WgT", [D, D], BF16, kind="ExternalInput").ap()
    WoT = nc.dram_tensor("WoT", [D, D], BF16, kind="ExternalInput").ap()
    bqs = nc.dram_tensor("bqs", [DP], FP32, kind="ExternalInput").ap()
    out = nc.dram_tensor("out", [RPC, D], FP32, kind="ExternalOutput").ap()

    with tile.TileContext(nc) as tc, ExitStack() as ctx:
        consts = ctx.enter_context(tc.tile_pool(name="consts", bufs=1))
        dram = ctx.enter_context(tc.tile_pool(name="dram", bufs=1, space="DRAM"))

        w01_sb = consts.tile([128, 2, 32], F8, name="w01_sb")
        nc.sync.dma_start(w01_sb[:], w01[:])
        sTc_sb = consts.tile([128, 6, RPC], BF16, name="sTc_sb")
        nc.scalar.dma_start(sTc_sb[:], sTc.rearrange("(a p) n -> p a n", p=128))
        wq_sb = consts.tile([128, 6, DP], BF16, name="wq_sb")
        nc.scalar.dma_start(wq_sb[:], WqT.rearrange("(a p) n -> p a n", p=128))
        wk_sb = consts.tile([128, 6, DP], BF16, name="wk_sb")
        nc.gpsimd.dma_start(wk_sb[:], WkT.rearrange("(a p) n -> p a n", p=128))
        wv_sb = consts.tile([128, 6, DP], BF16, name="wv_sb")
        nc.gpsimd.dma_start(wv_sb[:], WvT.rearrange("(a p) n -> p a n", p=128))
        wg_sb = consts.tile([128, 6, D], BF16, name="wg_sb")
        nc.scalar.dma_start(wg_sb[:], WgT.rearrange("(a p) n -> p a n", p=128))
        wo_sb = consts.tile([128, 6, D], BF16, name="wo_sb")
        nc.scalar.dma_start(wo_sb[:], WoT.rearrange("(a p) n -> p a n", p=128))
        bq_sb = consts.tile([128, 8], FP32, name="bq_sb")
        nc.sync.dma_start(bq_sb[:], bqs.rearrange("(b p) -> p b", p=128))
        ident = consts.tile([128, 128], BF16, name="ident")
        make_identity(nc, ident[:])
        eps_sb = consts.tile([128, 1], FP32, name="eps_sb")
        nc.vector.memset(eps_sb[:], EPS_S)

        kT_sb = consts.tile([128, 8, S], BF16, name="kT_sb")
        v_sb = consts.tile([128, 8, H, HDP + 1], BF16, name="v_sb")
        qT_sb = consts.tile([128, 8, RPC], BF16, name="qT_sb")
        g_sb = consts.tile([128, D], BF16, name="g_sb")
        oall = consts.tile([128, D], BF16, name="oall")
        mu_sb = consts.tile([128, S], BF16, name="mu_sb")
        ez2_sb = consts.tile([128, S], BF16, name="ez2_sb")
        r_sb = consts.tile([128, S], BF16, name="r_sb")
        var_sb = consts.tile([128, S], FP32, name="var_sb")

        y_flat = dram.tile([32 * RPC, S], BF16)

        nc.vector.memset(v_sb[:, :, :, HDP:HDP + 1], 1.0)

        # ---- stage B (projections) + stage C (pair-bias) share pools so
        # the scheduler can overlap z streaming with projection matmuls ----
        def square(eng, dst, src):
            if eng is nc.scalar:
                eng.activation(dst, src, AF.Square)
            else:
                eng.tensor_tensor(dst, src, src, ALU.mult)

        def copy(eng, dst, src):
            if eng is nc.scalar:
                eng.copy(dst, src)
            else:
                eng.tensor_copy(dst, src)

        # squares: Pool takes 11 groups (SBUF-only; GPSIMD cannot touch PSUM),
        # ACT 11, DVE 10 — balanced against each engine's PSUM-op load.
        sq_eng = []
        for g in range(RPC // 4):
            if g % 3 == 0:
                sq_eng.append(nc.gpsimd)
            else:
                sq_eng.append(nc.vector if (g % 2) else nc.scalar)
        with tc.tile_pool(name="psA", bufs=2, space="PSUM") as psA, \
             tc.tile_pool(name="psY", bufs=3, space="PSUM") as psY, \
             tc.tile_pool(name="zwork", bufs=3) as zw, \
             tc.tile_pool(name="ypool", bufs=2) as yp:
            # ---- stage C: fp8 DoubleRow pair-bias over own z rows ----
            # DoubleRow requires tile_position (0,0) + out at partition 0,
            # so each row gets its own [32, S] 2-bank PSUM tile; a per-row
            # engine copy packs 4 rows into one [128, S] bf16 tile that
            # leaves via a single bulk DMA per group.  z streams in 8-row
            # granules to halve DMA trigger count.
            for grp in range(RPC // 8):
                zq = zw.tile([128, 8, 2, S], F8, tag="zq")
                nc.sync.dma_start(zq[:, :, 0, :], zb[:, ds(8 * grp, 8), :])
                square(sq_eng[2 * grp], zq[:, 0:4, 1, :], zq[:, 0:4, 0, :])
                square(sq_eng[2 * grp + 1], zq[:, 4:8, 1, :], zq[:, 4:8, 0, :])
                for half in range(2):
                    y4 = yp.tile([128, S], BF16, tag="y4")
                    for j in range(4):
                        jj = 4 * half + j
                        ps = psY.tile([32, S], FP32, tag="ps")
                        for m in range(4):
                            nc.tensor.matmul(
                                ps[:, ds(256 * m, 256)],
                                lhsT=w01_sb[:],
                                rhs=zq[:, jj, :, ds(256 * m, 256)],
                                start=True, stop=True, perf_mode=DR,
                                tile_position=(0, 0))
                        copy(nc.vector if (grp + j) % 2 else nc.scalar,
                             y4[ds(32 * j, 32), :], ps[:])
                    nc.sync.dma_start(
                        y_flat[ds(128 * (2 * grp + half), 128)], y4[:])

            # ---- stage B: K/V sharded over cores (own 128-t slice each),
            # exchanged with a single fused AllGather ----
            kvs = consts.tile([128, 2048], BF16, name="kvs")
            kv_in = dram.tile([128, 2048], BF16)
            kv_out = dram.tile([N_CORES, 128, 2048], BF16)
            # kT shard: [dout_block, own 128 t]
            for blk in range(8):
                p = psA.tile([128, 512], FP32, tag="pA", name="pK")[:, :128]
                for ko in range(6):
                    nc.tensor.matmul(p[:], lhsT=wk_sb[:, ko, ts(blk, 128)],
                                     rhs=sTc_sb[:, ko, :],
                                     start=(ko == 0), stop=(ko == 5))
                copy(nc.scalar, kvs[:, ds(128 * blk, 128)], p[:])
            # v shard: [own 128 t, dout]
            for ch in range(2):
                p = psA.tile([128, 512], FP32, tag="pA")
                for ko in range(6):
                    nc.tensor.matmul(p[:], lhsT=sTc_sb[:, ko, :],
                                     rhs=wv_sb[:, ko, ts(ch, 512)],
                                     start=(ko == 0), stop=(ko == 5))
                copy(nc.vector, kvs[:, ds(1024 + 512 * ch, 512)], p[:])
            nc.scalar.dma_start(kv_in[:], kvs[:])
            nc.gpsimd.collective_compute(
                "AllGather", ALU.bypass,
                replica_groups=[list(range(N_CORES))],
                ins=[kv_in.opt()], outs=[kv_out.opt()])
            for c in range(N_CORES):
                (nc.sync if c % 2 else nc.scalar).dma_start(
                    kT_sb[:, :, ds(128 * c, 128)],
                    kv_out[c, :, 0:1024].rearrange("p (b t) -> p b t", b=8))
                (nc.scalar if c % 2 else nc.sync).dma_start(
                    v_sb[:, c, :, 0:HDP],
                    kv_out[c, :, 1024:2048].rearrange("p (h e) -> p h e", h=H))
            # qT for own rows, scaled by 1/sqrt(hd), bias added
            for blk in range(8):
                p = psA.tile([128, 512], FP32, tag="pA", name="pQ")[:, :RPC]
                for ko in range(6):
                    nc.tensor.matmul(p[:], lhsT=wq_sb[:, ko, ts(blk, 128)],
                                     rhs=sTc_sb[:, ko, :],
                                     start=(ko == 0), stop=(ko == 5))
                nc.scalar.activation(qT_sb[:, blk, :], p[:], AF.Identity,
                                     bias=bq_sb[:, blk:blk + 1], scale=ISQ)
            # g for own rows
            for ch, w in ((0, 512), (1, 256)):
                p = psA.tile([128, 512], FP32, tag="pA")
                for ko in range(6):
                    nc.tensor.matmul(p[:, :w], lhsT=sTc_sb[:, ko, :],
                                     rhs=wg_sb[:, ko, ds(512 * ch, w)],
                                     start=(ko == 0), stop=(ko == 5))
                nc.vector.tensor_copy(g_sb[:, ds(512 * ch, w)], p[:, :w])

        # ---- stage D: r from round-tripped stats (all 64-scaled) ----
        y_rows = y_flat.rearrange("(p a) t -> p a t", a=32)
        nc.sync.dma_start(mu_sb[:], y_rows[:, 16, :])
        nc.sync.dma_start(ez2_sb[:], y_rows[:, 17, :])
        nc.vector.tensor_tensor(var_sb[:], mu_sb[:], mu_sb[:], ALU.mult)
        nc.vector.scalar_tensor_tensor(var_sb[:], ez2_sb[:], WS, var_sb[:],
                                       op0=ALU.mult, op1=ALU.subtract)
        nc.scalar.activation(var_sb[:], var_sb[:], AF.Ln, bias=eps_sb[:])
        nc.scalar.activation(r_sb[:], var_sb[:], AF.Exp, scale=-0.5)

        # ---- stage E: attention per head ----
        with tc.tile_pool(name="psE", bufs=2, space="PSUM") as psE, \
             tc.tile_pool(name="head", bufs=2) as hw_pool:
            for h in range(H):
                po2, blk = 64 * (h % 2), h // 2
                y_h = hw_pool.tile([128, S], BF16, tag="yh")
                nc.sync.dma_start(y_h[:], y_rows[:, h, :])
                t1 = hw_pool.tile([128, S], BF16, tag="t1")
                nc.vector.tensor_tensor(t1[:], y_h[:], r_sb[:], ALU.mult)
                sc = hw_pool.tile([128, S], BF16, tag="sc")
                pq = psE.tile([128, S], FP32, tag="qk")
                for ch in range(2):
                    nc.tensor.matmul(pq[:, ts(ch, 512)],
                                     lhsT=qT_sb[ds(po2, 64), blk, :],
                                     rhs=kT_sb[ds(po2, 64), blk, ts(ch, 512)],
                                     start=True, stop=True)
                nc.vector.tensor_tensor(sc[:], pq[:], t1[:], ALU.add)
                aT = hw_pool.tile([128, 8, 128], BF16, tag="aT")
                for half in range(2):
                    pt = psE.tile([128, 512], BF16, tag="pt")
                    for jj in range(4):
                        nc.tensor.transpose(pt[:, ts(jj, 128)],
                                            sc[:, ts(4 * half + jj, 128)],
                                            ident[:])
                    nc.scalar.activation(aT[:, ds(4 * half, 4), :],
                                         pt.rearrange("p (a b) -> p a b", a=4),
                                         AF.Exp)
                po = psE.tile([128, HDP + 1], FP32, tag="po")
                for tb in range(8):
                    nc.tensor.matmul(po[:], lhsT=aT[:, tb, :],
                                     rhs=v_sb[:, tb, h, :],
                                     start=(tb == 0), stop=(tb == 7))
                dr = hw_pool.tile([128, 1], FP32, tag="dr")
                nc.vector.reciprocal(dr[:], po[:, HDP:HDP + 1])
                nc.vector.tensor_scalar(oall[:, ds(HD * h, HD)], po[:, 0:HD],
                                        dr[:], None, op0=ALU.mult)

            # ---- stage F: gate + output projection ----
            sig = hw_pool.tile([128, D], BF16, tag="sig")
            nc.scalar.activation(sig[:], g_sb[:], AF.Sigmoid)
            og = hw_pool.tile([128, D], BF16, tag="og")
            nc.vector.tensor_tensor(og[:], oall[:], sig[:], ALU.mult)
            ogT = hw_pool.tile([128, 6, 128], BF16, tag="ogT")
            for half, n in ((0, 4), (1, 2)):
                pt = psE.tile([128, 512], BF16, tag="pt")
                for jj in range(n):
                    nc.tensor.transpose(pt[:, ts(jj, 128)],
                                        og[:, ts(4 * half + jj, 128)], ident[:])
                copy(nc.vector if half else nc.scalar,
                     ogT[:, ds(4 * half, n), :],
                     pt.rearrange("p (a b) -> p a b", a=4)[:, 0:n, :])
            out_sb = hw_pool.tile([128, D], FP32, tag="outsb")
            pf = psE.tile([128, S], FP32, tag="qk")
            for ch, w in ((0, 512), (1, 256)):
                for ko in range(6):
                    nc.tensor.matmul(pf[:, ds(512 * ch, w)], lhsT=ogT[:, ko, :],
                                     rhs=wo_sb[:, ko, ds(512 * ch, w)],
                                     start=(ko == 0), stop=(ko == 5))
            copy(nc.vector, out_sb[:], pf[:, :D])
            nc.sync.dma_start(out[:], out_sb[:])

    nc.compile()
    return nc


def _prep(inputs):
    bf = ml_dtypes.bfloat16
    f8 = ml_dtypes.float8_e4m3
    s = np.asarray(inputs["s"], np.float32)[0]
    z = np.asarray(inputs["z"], np.float32)[0]
    Wq = np.asarray(inputs["Wq"], np.float32)
    bq = np.asarray(inputs["bq"], np.float32)
    Wk = np.asarray(inputs["Wk"], np.float32)
    Wv = np.asarray(inputs["Wv"], np.float32)
    Wg = np.asarray(inputs["Wg"], np.float32)
    ln_w = np.asarray(inputs["ln_w"], np.float32)
    ln_b = np.asarray(inputs["ln_b"], np.float32)  # noqa: F841 (softmax-invariant)
    Wz = np.asarray(inputs["Wz"], np.float32)
    Wo = np.asarray(inputs["Wo"], np.float32)

    def pad_rows(W):
        Wp = np.zeros((DP, D), np.float32)
        for h in range(H):
            Wp[h * HDP:h * HDP + HD] = W[h * HD:(h + 1) * HD]
        return Wp

    z8 = z.astype(f8)                            # [S, S, DZ]
    sT = np.ascontiguousarray(s.T).astype(bf)
    WqTp = np.ascontiguousarray(pad_rows(Wq).T).astype(bf)
    WkTp = np.ascontiguousarray(pad_rows(Wk).T).astype(bf)
    WvTp = np.ascontiguousarray(pad_rows(Wv).T).astype(bf)
    WgT = np.ascontiguousarray(Wg.T).astype(bf)
    WoT = np.ascontiguousarray(Wo.T).astype(bf)
    bq_p = np.zeros(DP, np.float32)
    for h in range(H):
        bq_p[h * HDP:h * HDP + HD] = bq[h * HD:(h + 1) * HD]
    bq_p *= ISQ

    Wzp = ln_w[None, :] * Wz                     # [H, DZ]
    c1 = Wzp.sum(-1)                             # [H]
    w01 = np.zeros((DZ, 2, 32), np.float32)
    w01[:, 0, :H] = (Wzp - c1[:, None] / DZ).T   # mean-fold baked in
    w01[:, 0, 16] = 1.0 / DZ                     # 64*mu column (after WS)
    w01[:, 1, 17] = 1.0 / DZ                     # 64*E[z^2] column
    w01 *= WS

    shared = {
        "WqT": WqTp, "WkT": WkTp, "WvT": WvTp, "WgT": WgT,
        "WoT": WoT, "bqs": bq_p, "w01": w01.astype(f8),
    }
    in_maps = []
    for ci in range(N_CORES):
        rows = slice(ci * RPC, (ci + 1) * RPC)
        m = dict(shared)
        m["zb"] = np.ascontiguousarray(z8[rows].transpose(2, 0, 1))
        m["sTc"] = np.ascontiguousarray(sT[:, rows])
        in_maps.append(m)
    return in_maps


def _install_ntff_hook():
    try:
        import antenv
        from trn_agent_boot.trn_boot import _ntff_profile_via_ctypes
        from concourse import bass_utils
        mod = types.ModuleType("antenv.axon_hooks")
        mod._hook = _ntff_profile_via_ctypes('/opt/axon/libaxon_pjrt.so')
        mod.set_axon_ntff_profile_hook = lambda h: setattr(mod, "_hook", h)
        mod.get_axon_ntff_profile_hook = lambda: mod._hook
        sys.modules["antenv.axon_hooks"] = mod
        antenv.axon_hooks = mod
        bass_utils.upload_artifacts = lambda tmpdir: tmpdir
    except Exception as e:  # profiling is best-effort
        print(f"ntff hook install failed: {e}", file=sys.stderr)


def run(inputs, trace=False):
    from concourse.bass_utils import run_bass_kernel_spmd
    in_maps = _prep(inputs)
    if "nc" not in _CACHE:
        _CACHE["nc"] = _build()
    nc = _CACHE["nc"]
    if trace:
        _install_ntff_hook()
    res = run_bass_kernel_spmd(nc, in_maps, core_ids=list(range(N_CORES)),
                               trace=trace)
    out = np.concatenate([res.results[i]["out"] for i in range(N_CORES)], axis=0)
    return out[None].astype(np.float32), res


def kernel(**inputs) -> np.ndarray:
    out, _ = run(inputs, trace=bool(os.environ.get("KERNEL_TRACE")))
    return out
